# revision 38
# baseline (speedup 1.0000x reference)
"""Distributed Trainium2 kernel for GQA attention (nn_Attention_76845554860188).

B=1, S=2048, D=1024, NH=16, NKV=4, HD=64, causal, RoPE, 8 NeuronCores.

Sharding: tensor-parallel over heads. Core c owns q-heads {2c, 2c+1} and their
(shared, GQA) kv-head c//2. Each core projects Q/K/V for all 2048 positions,
runs causal attention for its 2 heads, then an AllToAll redistributes the
per-head outputs into per-sequence shards; each core applies the full output
projection to its 256-row slice. Host concatenates the 8 slices.

On-chip layout is [feature, seq] ("transposed"): scores are computed with keys
on the partition axis, so the softmax denominator comes for free as a ones
column appended to V in the PV matmul. exp() runs on ScalarE with the 1/sqrt(64)
scale folded in; no max-subtraction is needed (logits are O(5) for unit-scale
inputs, far from fp32/bf16 overflow).
"""

import sys

sys.path.insert(0, "/opt/trn_rl_repo")

import numpy as np
import ml_dtypes

import concourse.bass as bass
import concourse.mybir as mybir
import concourse.tile as tile
from concourse import bacc
from concourse.bass_utils import run_bass_kernel_spmd

BF16 = mybir.dt.bfloat16
F32 = mybir.dt.float32

B, S, D = 1, 2048, 1024
NH, NKV, HD = 16, 4, 64
NC_CORES = 8
HPC = NH // NC_CORES  # q heads per core = 2
SC = S // NC_CORES  # seq slice per core = 256
NDC = D // 128  # d chunks = 8
NSB = S // 128  # seq blocks = 16
HALF = HD // 2  # 32

np_bf16 = ml_dtypes.bfloat16


def build_graph(taps=False):
    nc = bacc.Bacc(
        "TRN2", target_bir_lowering=False, debug=False, num_devices=NC_CORES
    )

    # ---- DRAM parameters (per-core shards supplied by host) ----
    xT_e = nc.dram_tensor("xT", [D, S], BF16, kind="ExternalInput")
    wq_e = nc.dram_tensor("wq", [D, HPC * HD], BF16, kind="ExternalInput")
    wkv_e = nc.dram_tensor("wkv", [D, 2 * HD], BF16, kind="ExternalInput")
    wo_e = nc.dram_tensor("wo", [D, D], BF16, kind="ExternalInput")
    c2_e = nc.dram_tensor("c2", [128, S], BF16, kind="ExternalInput")
    s2_e = nc.dram_tensor("s2", [128, S], BF16, kind="ExternalInput")
    ppm_e = nc.dram_tensor("ppm", [128, 128], BF16, kind="ExternalInput")
    idm_e = nc.dram_tensor("idm", [128, 128], BF16, kind="ExternalInput")
    tri_e = nc.dram_tensor("tri", [128, 128], BF16, kind="ExternalInput")
    out_e = nc.dram_tensor("out", [SC, D], F32, kind="ExternalOutput")

    # internal DRAM bounce buffers for the per-seq-chunk AllGathers
    send_d = [nc.dram_tensor(f"ag_send{k}", [128, SC], BF16) for k in range(NC_CORES)]
    recv_d = nc.dram_tensor(
        "ag_recv", [NC_CORES, NC_CORES, 128, SC], BF16, addr_space="Shared"
    )

    tap_tensors = None
    if taps:
        tap_tensors = {
            "tap_qrot0": nc.dram_tensor("tap_qrot0", [64, S], BF16, kind="ExternalOutput"),
            "tap_qrot1": nc.dram_tensor("tap_qrot1", [64, S], BF16, kind="ExternalOutput"),
            "tap_krot": nc.dram_tensor("tap_krot", [64, S], BF16, kind="ExternalOutput"),
            "tap_vext": nc.dram_tensor("tap_vext", [128, NSB, HD + 1], BF16, kind="ExternalOutput"),
            "tap_pt0": nc.dram_tensor("tap_pt0", [128, S], BF16, kind="ExternalOutput"),
            "tap_ot0": nc.dram_tensor("tap_ot0", [HD + 1, S], F32, kind="ExternalOutput"),
            "tap_stage": nc.dram_tensor("tap_stage", [128, S], BF16, kind="ExternalOutput"),
            "tap_at": nc.dram_tensor("tap_at", [128, NC_CORES, SC], BF16, kind="ExternalOutput"),
        }

    with tile.TileContext(nc) as tc:
        _body(nc, tc, xT_e, wq_e, wkv_e, wo_e, c2_e, s2_e, ppm_e, idm_e, tri_e,
              out_e, send_d, recv_d, tap_tensors)

    nc.compile()
    return nc


def _body(nc, tc, xT_e, wq_e, wkv_e, wo_e, c2_e, s2_e, ppm_e, idm_e, tri_e,
          out_e, send_d, recv_d, taps=None):
    from contextlib import ExitStack

    ctx = ExitStack()
    with ctx:
        consts = ctx.enter_context(tc.tile_pool(name="consts", bufs=1))
        work = ctx.enter_context(tc.tile_pool(name="work", bufs=1))
        rope_cm = tc.tile_pool(name="rope", bufs=1)
        rope = rope_cm.__enter__()
        psum_cm = tc.tile_pool(name="psum", bufs=2, space="PSUM")
        psum = psum_cm.__enter__()

        # ---- load inputs needed by the preamble (Wo is loaded later) ----
        wq_sb = consts.tile([128, NDC, HPC * HD], BF16, tag="wq")
        nc.scalar.dma_start(
            out=wq_sb[:], in_=wq_e.ap().rearrange("(i p) h -> p i h", p=128)
        )
        wkv_sb = consts.tile([128, NDC, 2 * HD], BF16, tag="wkv")
        nc.scalar.dma_start(
            out=wkv_sb[:], in_=wkv_e.ap().rearrange("(i p) h -> p i h", p=128)
        )
        xT_sb = consts.tile([128, NDC, S], BF16, tag="xT")
        for i in range(NDC):
            eng = nc.sync if i % 2 == 0 else nc.scalar
            eng.dma_start(out=xT_sb[:, i, :], in_=xT_e[128 * i : 128 * (i + 1), :])
        c2_sb = rope.tile([128, S], BF16, tag="c2")
        nc.sync.dma_start(out=c2_sb[:], in_=c2_e[:, :])
        s2_sb = rope.tile([128, S], BF16, tag="s2")
        nc.scalar.dma_start(out=s2_sb[:], in_=s2_e[:, :])
        ppm_sb = rope.tile([128, 128], BF16, tag="ppm")
        nc.sync.dma_start(out=ppm_sb[:], in_=ppm_e[:, :])
        idm_sb = rope.tile([128, 128], BF16, tag="idm")
        nc.sync.dma_start(out=idm_sb[:], in_=idm_e[:, :])
        tri_sb = consts.tile([128, 128], BF16, tag="tri")
        nc.sync.dma_start(out=tri_sb[:], in_=tri_e[:, :])

        NCH = S // 512  # 512-wide chunks across seq

        # ---- Q/KV projections -> PSUM f32 [128, 2048] ----
        q_ps = psum.tile([128, S], F32, tag="big")
        for i in range(NDC):
            for n in range(NCH):
                nc.tensor.matmul(
                    q_ps[:, 512 * n : 512 * (n + 1)],
                    lhsT=wq_sb[:, i, :],
                    rhs=xT_sb[:, i, 512 * n : 512 * (n + 1)],
                    start=(i == 0),
                    stop=(i == NDC - 1),
                )
        kv_ps = psum.tile([128, S], F32, tag="big")
        for i in range(NDC):
            for n in range(NCH):
                nc.tensor.matmul(
                    kv_ps[:, 512 * n : 512 * (n + 1)],
                    lhsT=wkv_sb[:, i, :],
                    rhs=xT_sb[:, i, 512 * n : 512 * (n + 1)],
                    start=(i == 0),
                    stop=(i == NDC - 1),
                )

        # ---- RoPE on q (both heads at once) ----
        # qrot = (q .* C2) + M @ (q .* S2); lhsT ppm = M.T, idm = I.
        qc_sb = rope.tile([128, S], BF16, tag="qc")
        nc.vector.tensor_tensor(
            out=qc_sb[:], in0=q_ps[:], in1=c2_sb[:], op=mybir.AluOpType.mult
        )
        qs_sb = rope.tile([128, S], BF16, tag="qs")
        nc.vector.tensor_tensor(
            out=qs_sb[:], in0=q_ps[:], in1=s2_sb[:], op=mybir.AluOpType.mult
        )
        qrot_ps = psum.tile([128, S], F32, tag="big")
        for n in range(NCH):
            sl = slice(512 * n, 512 * (n + 1))
            nc.tensor.matmul(
                qrot_ps[:, sl], lhsT=ppm_sb[:], rhs=qs_sb[:, sl], start=True, stop=False
            )
            nc.tensor.matmul(
                qrot_ps[:, sl], lhsT=idm_sb[:], rhs=qc_sb[:, sl], start=False, stop=True
            )
        # per-head tiles (base partition 0) since matmul requires matching bases
        qrot_h = []
        for h in range(HPC):
            qr = work.tile([64, S], BF16, tag=f"qrot{h}")
            nc.scalar.copy(out=qr[:], in_=qrot_ps[64 * h : 64 * (h + 1), :])
            qrot_h.append(qr)
        if taps:
            nc.sync.dma_start(out=taps["tap_qrot0"].ap(), in_=qrot_h[0][:])
            nc.sync.dma_start(out=taps["tap_qrot1"].ap(), in_=qrot_h[1][:])

        # ---- RoPE on k (rows 0:64 of kv), V extraction (rows 64:128) ----
        kc_sb = rope.tile([64, S], BF16, tag="kc")
        nc.vector.tensor_tensor(
            out=kc_sb[:], in0=kv_ps[0:64, :], in1=c2_sb[0:64, :],
            op=mybir.AluOpType.mult,
        )
        ks_sb = rope.tile([64, S], BF16, tag="ks")
        nc.vector.tensor_tensor(
            out=ks_sb[:], in0=kv_ps[0:64, :], in1=s2_sb[0:64, :],
            op=mybir.AluOpType.mult,
        )
        vT_sb = rope.tile([64, S], BF16, tag="vT")
        nc.vector.tensor_copy(out=vT_sb[:], in_=kv_ps[64:128, :])

        krot_ps = psum.tile([128, S], F32, tag="big")
        for n in range(NCH):
            sl = slice(512 * n, 512 * (n + 1))
            nc.tensor.matmul(
                krot_ps[0:64, sl], lhsT=ppm_sb[0:64, 0:64], rhs=ks_sb[:, sl],
                start=True, stop=False,
            )
            nc.tensor.matmul(
                krot_ps[0:64, sl], lhsT=idm_sb[0:64, 0:64], rhs=kc_sb[:, sl],
                start=False, stop=True,
            )
        krot_sb = work.tile([64, S], BF16, tag="krot")
        nc.scalar.copy(out=krot_sb[:], in_=krot_ps[0:64, :])
        if taps:
            nc.sync.dma_start(out=taps["tap_krot"].ap(), in_=krot_sb[:])

        # ---- V transpose: vT [64, S] -> V blocks [128, 64] + ones col ----
        vext_sb = work.tile([128, NSB, HD + 1], BF16, tag="vext")
        nc.vector.memset(vext_sb[:, :, HD : HD + 1], 1.0)
        vt_ps = psum.tile([128, NSB, HD], BF16, tag="big")
        for b in range(NSB):
            nc.tensor.transpose(
                vt_ps[:, b, :], vT_sb[:, 128 * b : 128 * (b + 1)], idm_sb[0:64, 0:64]
            )
        nc.vector.tensor_copy(out=vext_sb[:, :, 0:HD], in_=vt_ps[:])
        if taps:
            nc.sync.dma_start(out=taps["tap_vext"].ap(), in_=vext_sb[:])

        # release RoPE temporaries so their SBUF range can host the PT pool;
        # release the proj-phase PSUM pool so attention can use its own layout
        rope_cm.__exit__(None, None, None)
        psum_cm.__exit__(None, None, None)
        ptp = ctx.enter_context(tc.tile_pool(name="pt", bufs=2, space="SBUF"))
        psa_cm = tc.tile_pool(name="psa", bufs=2, space="PSUM")
        psa = psa_cm.__enter__()

        # ---- attention: sq-chunked; chunk k (both heads) is final after
        # k-block 2k+1 thanks to causality, so its AllGather overlaps the rest
        scale = 1.0 / np.sqrt(HD)
        blk_w = [S - 128 * b for b in range(NSB)]  # ragged valid width per k-block
        blk_off = [0] * NSB
        for b in range(1, NSB):
            blk_off[b] = blk_off[b - 1] + blk_w[b - 1]
        pt_total = blk_off[-1] + blk_w[-1]

        pt_h = [
            ptp.tile([128, pt_total], BF16, tag="ptbuf", name=f"pt_h{h}")
            for h in range(HPC)
        ]

        for b in range(NSB):
            w = blk_w[b]
            kb = krot_sb[:, 128 * b : 128 * (b + 1)]
            for h in range(HPC):
                qh = qrot_h[h][:]
                for n2 in range((w + 1023) // 1024):
                    cw2 = min(1024, w - 1024 * n2)
                    st_ps = psa.tile([128, 1024], F32, tag="st")  # bufs=2 below
                    for n in range((cw2 + 511) // 512):
                        cw = min(512, cw2 - 512 * n)
                        q0 = 128 * b + 1024 * n2 + 512 * n
                        nc.tensor.matmul(
                            st_ps[:, 512 * n : 512 * n + cw],
                            lhsT=kb,
                            rhs=qh[:, q0 : q0 + cw],
                            start=True,
                            stop=True,
                        )
                    nc.scalar.activation(
                        out=pt_h[h][:, blk_off[b] + 1024 * n2 : blk_off[b] + 1024 * n2 + cw2],
                        in_=st_ps[:, 0:cw2],
                        func=mybir.ActivationFunctionType.Exp,
                        scale=scale,
                    )
                # mask the diagonal 128x128 sub-block (sq < sk -> 0)
                nc.vector.tensor_tensor(
                    out=pt_h[h][:, blk_off[b] : blk_off[b] + 128],
                    in0=pt_h[h][:, blk_off[b] : blk_off[b] + 128],
                    in1=tri_sb[:],
                    op=mybir.AluOpType.mult,
                )

            if b % 2 == 0:
                continue
            # chunk k = (b-1)/2 is complete: PV + normalize + AllGather it
            k = b // 2
            base = SC * k
            stg = work.tile([128, SC], BF16, tag="stg")
            for h in range(HPC):
                ot_ps = psa.tile([128, SC], F32, tag="ot")
                for b2 in range(b + 1):
                    lo = max(base, 128 * b2)
                    hi = base + SC
                    nc.tensor.matmul(
                        ot_ps[0 : HD + 1, lo - base : hi - base],
                        lhsT=vext_sb[:, b2, :],
                        rhs=pt_h[h][
                            :, blk_off[b2] + lo - 128 * b2 : blk_off[b2] + hi - 128 * b2
                        ],
                        start=(b2 == 0),
                        stop=(b2 == b),
                    )
                den_sb = work.tile([1, SC], F32, tag="den")
                nc.vector.tensor_copy(out=den_sb[:], in_=ot_ps[HD : HD + 1, :])
                rec_sb = work.tile([1, SC], F32, tag="rec")
                nc.vector.reciprocal_approx_fast(out=rec_sb[:], in_=den_sb[:])
                bcr_sb = work.tile([HD, SC], F32, tag="bcr")
                nc.gpsimd.partition_broadcast(bcr_sb[:], rec_sb[:])
                nc.vector.tensor_tensor(
                    out=stg[64 * h : 64 * (h + 1), :],
                    in0=ot_ps[0:HD, :],
                    in1=bcr_sb[:],
                    op=mybir.AluOpType.mult,
                )
            nc.sync.dma_start(out=send_d[k].ap(), in_=stg[:])
            nc.gpsimd.collective_compute(
                "AllGather",
                mybir.AluOpType.bypass,
                replica_groups=[list(range(NC_CORES))],
                ins=[send_d[k].ap().opt()],
                outs=[recv_d.ap()[k]],
            )

        # ---- rank-dynamic slice: this core's seq chunk, all 16 heads ----
        psa_cm.__exit__(None, None, None)
        pso = ctx.enter_context(tc.tile_pool(name="pso", bufs=1, space="PSUM"))

        # Wo load: deferred past the preamble so it doesn't compete with xT
        wo_sb = consts.tile([128, NDC, D], BF16, tag="wo")
        for i in range(NDC):
            eng = nc.sync if i % 2 == 0 else nc.scalar
            eng.dma_start(out=wo_sb[:, i, :], in_=wo_e[128 * i : 128 * (i + 1), :])
        rank = nc.partition_id()
        at_sb = work.tile([128, NC_CORES, SC], BF16, tag="at")
        nc.sync.dma_start(
            out=at_sb[:],
            in_=recv_d.ap()[bass.ds(rank, 1)].rearrange("k j p w -> p (k j) w"),
        )
        if taps:
            nc.sync.dma_start(out=taps["tap_at"].ap(), in_=at_sb[:])

        # ---- output projection: out[s, :] = attn[s, :] @ Wo ----
        op_ps = pso.tile([128, S], F32, tag="op")  # [128, 2 m x 1024 dout]
        for m in range(SC // 128):
            for j in range(NC_CORES):
                for dn in range(2):
                    nc.tensor.matmul(
                        op_ps[:, 1024 * m + 512 * dn : 1024 * m + 512 * (dn + 1)],
                        lhsT=at_sb[:, j, 128 * m : 128 * (m + 1)],
                        rhs=wo_sb[:, j, 512 * dn : 512 * (dn + 1)],
                        start=(j == 0),
                        stop=(j == NC_CORES - 1),
                    )
        out_sb = work.tile([128, S], F32, tag="osb")
        nc.scalar.copy(out=out_sb[:], in_=op_ps[:])
        nc.sync.dma_start(
            out=out_e.ap().rearrange("(m p) d -> p m d", p=128),
            in_=out_sb[:].rearrange("p (m d) -> p m d", m=SC // 128),
        )


# ---------------- host side ----------------

_CACHE = {}


def _prep_consts():
    # ppm: lhsT of the signed half-swap M (per 64 block: [[0,-I],[I,0]])
    M = np.zeros((128, 128), np.float32)
    for hb in range(2):
        o = 64 * hb
        for j in range(HALF):
            M[o + j, o + HALF + j] = -1.0
            M[o + HALF + j, o + j] = 1.0
    ppm = M.T.astype(np_bf16)
    idm = np.eye(128, dtype=np_bf16)
    # tri[p, j] = 1 if j >= p (valid: sq >= sk within diagonal block)
    tri = (np.arange(128)[None, :] >= np.arange(128)[:, None]).astype(np_bf16)
    return ppm, idm, tri


def kernel(x, rope_cos, rope_sin, Wq, Wk, Wv, Wo):
    if "nc" not in _CACHE:
        _CACHE["nc"] = build_graph()
    nc = _CACHE["nc"]

    x2 = np.asarray(x, np.float32).reshape(S, D)
    xT = np.ascontiguousarray(x2.T).astype(np_bf16)
    cosT = np.asarray(rope_cos, np.float32).T  # [32, S]
    sinT = np.asarray(rope_sin, np.float32).T
    c2 = np.tile(cosT, (4, 1)).astype(np_bf16)  # [128, S]
    s2 = np.tile(sinT, (4, 1)).astype(np_bf16)
    ppm, idm, tri = _prep_consts()

    Wq = np.asarray(Wq, np.float32)
    Wk = np.asarray(Wk, np.float32)
    Wv = np.asarray(Wv, np.float32)
    Wo = np.asarray(Wo, np.float32)
    wo_b = Wo.astype(np_bf16)

    in_maps = []
    for c in range(NC_CORES):
        kv = c // 2
        wq_c = Wq[:, HPC * HD * c : HPC * HD * (c + 1)].astype(np_bf16)
        wkv_c = np.concatenate(
            [Wk[:, HD * kv : HD * (kv + 1)], Wv[:, HD * kv : HD * (kv + 1)]], axis=1
        ).astype(np_bf16)
        in_maps.append(
            {
                "xT": xT,
                "wq": np.ascontiguousarray(wq_c),
                "wkv": np.ascontiguousarray(wkv_c),
                "wo": wo_b,
                "c2": c2,
                "s2": s2,
                "ppm": ppm,
                "idm": idm,
                "tri": tri,
            }
        )

    res = run_bass_kernel_spmd(nc, in_maps, core_ids=list(range(NC_CORES)))
    out = np.concatenate([res.results[c]["out"] for c in range(NC_CORES)], axis=0)
    return out.reshape(B, S, D).astype(np.float32)


# revision 50
# speedup vs baseline: 1.0744x; 1.0744x over previous
"""Distributed Trainium2 kernel for GQA attention (nn_Attention_76845554860188).

B=1, S=2048, D=1024, NH=16, NKV=4, HD=64, causal, RoPE, 8 NeuronCores.

Sharding: tensor-parallel over heads. Core c owns q-heads {2c, 2c+1} and their
(shared, GQA) kv-head c//2. Each core projects Q/K/V for all 2048 positions,
runs causal attention for its 2 heads, then an AllToAll redistributes the
per-head outputs into per-sequence shards; each core applies the full output
projection to its 256-row slice. Host concatenates the 8 slices.

On-chip layout is [feature, seq] ("transposed"): scores are computed with keys
on the partition axis, so the softmax denominator comes for free as a ones
column appended to V in the PV matmul. exp() runs on ScalarE with the 1/sqrt(64)
scale folded in; no max-subtraction is needed (logits are O(5) for unit-scale
inputs, far from fp32/bf16 overflow).
"""

import sys

sys.path.insert(0, "/opt/trn_rl_repo")

import numpy as np
import ml_dtypes

import concourse.bass as bass
import concourse.mybir as mybir
import concourse.tile as tile
from concourse import bacc
from concourse.bass_utils import run_bass_kernel_spmd

BF16 = mybir.dt.bfloat16
F32 = mybir.dt.float32

B, S, D = 1, 2048, 1024
NH, NKV, HD = 16, 4, 64
NC_CORES = 8
HPC = NH // NC_CORES  # q heads per core = 2
SC = S // NC_CORES  # seq slice per core = 256
NDC = D // 128  # d chunks = 8
NSB = S // 128  # seq blocks = 16
HALF = HD // 2  # 32

np_bf16 = ml_dtypes.bfloat16


def build_graph(taps=False):
    nc = bacc.Bacc(
        "TRN2", target_bir_lowering=False, debug=False, num_devices=NC_CORES
    )

    # ---- DRAM parameters (per-core shards supplied by host) ----
    xT_e = nc.dram_tensor("xT", [D, S], BF16, kind="ExternalInput")
    # weights arrive pre-rearranged host-side into [partition, d-chunk, cols]
    wq_e = nc.dram_tensor("wq", [128, NDC, HPC * HD], BF16, kind="ExternalInput")
    wkv_e = nc.dram_tensor("wkv", [128, NDC, 2 * HD], BF16, kind="ExternalInput")
    wo_e = nc.dram_tensor("wo", [128, NDC, D], BF16, kind="ExternalInput")
    c2_e = nc.dram_tensor("c2", [128, S], BF16, kind="ExternalInput")
    s2_e = nc.dram_tensor("s2", [128, S], BF16, kind="ExternalInput")
    ppm_e = nc.dram_tensor("ppm", [128, 128], BF16, kind="ExternalInput")
    idm_e = nc.dram_tensor("idm", [128, 128], BF16, kind="ExternalInput")
    tri_e = nc.dram_tensor("tri", [128, 128], BF16, kind="ExternalInput")
    out_e = nc.dram_tensor("out", [SC, D], F32, kind="ExternalOutput")

    # internal DRAM bounce buffers for the per-seq-chunk AllGathers (4 chunks)
    send_d = [nc.dram_tensor(f"ag_send{k}", [128, 512], BF16) for k in range(4)]
    recv_d = nc.dram_tensor(
        "ag_recv", [4, NC_CORES, 128, 512], BF16, addr_space="Shared"
    )

    tap_tensors = None
    if taps:
        tap_tensors = {
            "tap_qrot0": nc.dram_tensor("tap_qrot0", [64, S], BF16, kind="ExternalOutput"),
            "tap_qrot1": nc.dram_tensor("tap_qrot1", [64, S], BF16, kind="ExternalOutput"),
            "tap_krot": nc.dram_tensor("tap_krot", [64, S], BF16, kind="ExternalOutput"),
            "tap_vext": nc.dram_tensor("tap_vext", [128, NSB, HD + 1], BF16, kind="ExternalOutput"),
            "tap_pt0": nc.dram_tensor("tap_pt0", [128, S], BF16, kind="ExternalOutput"),
            "tap_ot0": nc.dram_tensor("tap_ot0", [HD + 1, S], F32, kind="ExternalOutput"),
            "tap_stage": nc.dram_tensor("tap_stage", [128, S], BF16, kind="ExternalOutput"),
            "tap_at": nc.dram_tensor("tap_at", [128, NC_CORES, SC], BF16, kind="ExternalOutput"),
        }

    with tile.TileContext(nc) as tc:
        _body(nc, tc, xT_e, wq_e, wkv_e, wo_e, c2_e, s2_e, ppm_e, idm_e, tri_e,
              out_e, send_d, recv_d, tap_tensors)

    nc.compile()
    return nc


def _body(nc, tc, xT_e, wq_e, wkv_e, wo_e, c2_e, s2_e, ppm_e, idm_e, tri_e,
          out_e, send_d, recv_d, taps=None):
    from contextlib import ExitStack

    ctx = ExitStack()
    with ctx:
        consts = ctx.enter_context(tc.tile_pool(name="consts", bufs=1))
        work = ctx.enter_context(tc.tile_pool(name="work", bufs=1))
        rope_cm = tc.tile_pool(name="rope", bufs=1)
        rope = rope_cm.__enter__()
        psum_cm = tc.tile_pool(name="psum", bufs=2, space="PSUM")
        psum = psum_cm.__enter__()

        # ---- load inputs needed by the preamble (Wo is loaded later) ----
        wq_sb = consts.tile([128, NDC, HPC * HD], BF16, tag="wq")
        nc.scalar.dma_start(
            out=wq_sb[:], in_=wq_e.ap()
        )
        wkv_sb = consts.tile([128, NDC, 2 * HD], BF16, tag="wkv")
        nc.scalar.dma_start(
            out=wkv_sb[:], in_=wkv_e.ap()
        )
        xT_sb = consts.tile([128, NDC, S], BF16, tag="xT")
        for i in range(NDC):
            eng = nc.sync if i % 2 == 0 else nc.scalar
            eng.dma_start(out=xT_sb[:, i, :], in_=xT_e[128 * i : 128 * (i + 1), :])
        c2_sb = rope.tile([128, S], BF16, tag="c2")
        nc.sync.dma_start(out=c2_sb[:], in_=c2_e[:, :])
        s2_sb = rope.tile([128, S], BF16, tag="s2")
        nc.scalar.dma_start(out=s2_sb[:], in_=s2_e[:, :])
        ppm_sb = rope.tile([128, 128], BF16, tag="ppm")
        nc.sync.dma_start(out=ppm_sb[:], in_=ppm_e[:, :])
        idm_sb = rope.tile([128, 128], BF16, tag="idm")
        nc.sync.dma_start(out=idm_sb[:], in_=idm_e[:, :])
        tri_sb = consts.tile([128, 128], BF16, tag="tri")
        nc.sync.dma_start(out=tri_sb[:], in_=tri_e[:, :])

        NCH = S // 512  # 512-wide chunks across seq

        # ---- Q/KV projections -> PSUM f32 [128, 2048] ----
        q_ps = psum.tile([128, S], F32, tag="big")
        for i in range(NDC):
            for n in range(NCH):
                nc.tensor.matmul(
                    q_ps[:, 512 * n : 512 * (n + 1)],
                    lhsT=wq_sb[:, i, :],
                    rhs=xT_sb[:, i, 512 * n : 512 * (n + 1)],
                    start=(i == 0),
                    stop=(i == NDC - 1),
                )
        kv_ps = psum.tile([128, S], F32, tag="big")
        for i in range(NDC):
            for n in range(NCH):
                nc.tensor.matmul(
                    kv_ps[:, 512 * n : 512 * (n + 1)],
                    lhsT=wkv_sb[:, i, :],
                    rhs=xT_sb[:, i, 512 * n : 512 * (n + 1)],
                    start=(i == 0),
                    stop=(i == NDC - 1),
                )

        # ---- RoPE on q (both heads at once) ----
        # qrot = (q .* C2) + M @ (q .* S2); lhsT ppm = M.T, idm = I.
        qc_sb = rope.tile([128, S], BF16, tag="qc")
        nc.vector.tensor_tensor(
            out=qc_sb[:], in0=q_ps[:], in1=c2_sb[:], op=mybir.AluOpType.mult
        )
        qs_sb = rope.tile([128, S], BF16, tag="qs")
        nc.vector.tensor_tensor(
            out=qs_sb[:], in0=q_ps[:], in1=s2_sb[:], op=mybir.AluOpType.mult
        )
        qrot_ps = psum.tile([128, S], F32, tag="big")
        for n in range(NCH):
            sl = slice(512 * n, 512 * (n + 1))
            nc.tensor.matmul(
                qrot_ps[:, sl], lhsT=ppm_sb[:], rhs=qs_sb[:, sl], start=True, stop=False
            )
            nc.tensor.matmul(
                qrot_ps[:, sl], lhsT=idm_sb[:], rhs=qc_sb[:, sl], start=False, stop=True
            )
        # per-head tiles (base partition 0) since matmul requires matching bases
        qrot_h = []
        for h in range(HPC):
            qr = work.tile([64, S], BF16, tag=f"qrot{h}")
            nc.scalar.copy(out=qr[:], in_=qrot_ps[64 * h : 64 * (h + 1), :])
            qrot_h.append(qr)
        if taps:
            nc.sync.dma_start(out=taps["tap_qrot0"].ap(), in_=qrot_h[0][:])
            nc.sync.dma_start(out=taps["tap_qrot1"].ap(), in_=qrot_h[1][:])

        # ---- RoPE on k (rows 0:64 of kv), V extraction (rows 64:128) ----
        kc_sb = rope.tile([64, S], BF16, tag="kc")
        nc.vector.tensor_tensor(
            out=kc_sb[:], in0=kv_ps[0:64, :], in1=c2_sb[0:64, :],
            op=mybir.AluOpType.mult,
        )
        ks_sb = rope.tile([64, S], BF16, tag="ks")
        nc.vector.tensor_tensor(
            out=ks_sb[:], in0=kv_ps[0:64, :], in1=s2_sb[0:64, :],
            op=mybir.AluOpType.mult,
        )
        vT_sb = rope.tile([64, S], BF16, tag="vT")
        nc.vector.tensor_copy(out=vT_sb[:], in_=kv_ps[64:128, :])

        krot_ps = psum.tile([128, S], F32, tag="big")
        for n in range(NCH):
            sl = slice(512 * n, 512 * (n + 1))
            nc.tensor.matmul(
                krot_ps[0:64, sl], lhsT=ppm_sb[0:64, 0:64], rhs=ks_sb[:, sl],
                start=True, stop=False,
            )
            nc.tensor.matmul(
                krot_ps[0:64, sl], lhsT=idm_sb[0:64, 0:64], rhs=kc_sb[:, sl],
                start=False, stop=True,
            )
        krot_sb = work.tile([64, S], BF16, tag="krot")
        nc.scalar.copy(out=krot_sb[:], in_=krot_ps[0:64, :])
        if taps:
            nc.sync.dma_start(out=taps["tap_krot"].ap(), in_=krot_sb[:])

        # ---- V transpose: vT [64, S] -> V blocks [128, 64] + ones col ----
        vext_sb = work.tile([128, NSB, HD + 1], BF16, tag="vext")
        nc.vector.memset(vext_sb[:, :, HD : HD + 1], 1.0)
        vt_ps = psum.tile([128, NSB, HD], BF16, tag="big")
        for b in range(NSB):
            nc.tensor.transpose(
                vt_ps[:, b, :], vT_sb[:, 128 * b : 128 * (b + 1)], idm_sb[0:64, 0:64]
            )
        nc.vector.tensor_copy(out=vext_sb[:, :, 0:HD], in_=vt_ps[:])
        if taps:
            nc.sync.dma_start(out=taps["tap_vext"].ap(), in_=vext_sb[:])

        # release RoPE temporaries so their SBUF range can host the PT pool;
        # release the proj-phase PSUM pool so attention can use its own layout
        rope_cm.__exit__(None, None, None)
        psum_cm.__exit__(None, None, None)
        ptp = ctx.enter_context(tc.tile_pool(name="pt", bufs=2, space="SBUF"))
        psa_cm = tc.tile_pool(name="psa", bufs=2, space="PSUM")
        psa = psa_cm.__enter__()

        # ---- attention: sq-chunked; chunk k (both heads) is final after
        # k-block 2k+1 thanks to causality, so its AllGather overlaps the rest
        scale = 1.0 / np.sqrt(HD)
        blk_w = [S - 128 * b for b in range(NSB)]  # ragged valid width per k-block
        blk_off = [0] * NSB
        for b in range(1, NSB):
            blk_off[b] = blk_off[b - 1] + blk_w[b - 1]
        pt_total = blk_off[-1] + blk_w[-1]

        pt_h = [
            ptp.tile([128, pt_total], BF16, tag="ptbuf", name=f"pt_h{h}")
            for h in range(HPC)
        ]

        for b in range(NSB):
            w = blk_w[b]
            kb = krot_sb[:, 128 * b : 128 * (b + 1)]
            for h in range(HPC):
                qh = qrot_h[h][:]
                for n2 in range((w + 1023) // 1024):
                    cw2 = min(1024, w - 1024 * n2)
                    st_ps = psa.tile([128, 1024], F32, tag="st", bufs=3)
                    for n in range((cw2 + 511) // 512):
                        cw = min(512, cw2 - 512 * n)
                        q0 = 128 * b + 1024 * n2 + 512 * n
                        nc.tensor.matmul(
                            st_ps[:, 512 * n : 512 * n + cw],
                            lhsT=kb,
                            rhs=qh[:, q0 : q0 + cw],
                            start=True,
                            stop=True,
                        )
                    nc.scalar.activation(
                        out=pt_h[h][:, blk_off[b] + 1024 * n2 : blk_off[b] + 1024 * n2 + cw2],
                        in_=st_ps[:, 0:cw2],
                        func=mybir.ActivationFunctionType.Exp,
                        scale=scale,
                    )
                # mask the diagonal 128x128 sub-block (sq < sk -> 0)
                nc.vector.tensor_tensor(
                    out=pt_h[h][:, blk_off[b] : blk_off[b] + 128],
                    in0=pt_h[h][:, blk_off[b] : blk_off[b] + 128],
                    in1=tri_sb[:],
                    op=mybir.AluOpType.mult,
                )

            if b % 4 != 3:
                continue
            # chunk k = b//4 (sq 512-wide) is complete: PV + normalize + AllGather
            k = b // 4
            base = 512 * k
            stg = work.tile([128, 512], BF16, tag="stg")
            for h in range(HPC):
                ot_ps = psa.tile([128, 512], F32, tag="ot")
                for b2 in range(b + 1):
                    lo = max(base, 128 * b2)
                    hi = base + 512
                    nc.tensor.matmul(
                        ot_ps[0 : HD + 1, lo - base : hi - base],
                        lhsT=vext_sb[:, b2, :],
                        rhs=pt_h[h][
                            :, blk_off[b2] + lo - 128 * b2 : blk_off[b2] + hi - 128 * b2
                        ],
                        start=(b2 == 0),
                        stop=(b2 == b),
                    )
                den_sb = work.tile([1, 512], F32, tag="den")
                nc.vector.tensor_copy(out=den_sb[:], in_=ot_ps[HD : HD + 1, :])
                rec_sb = work.tile([1, 512], F32, tag="rec")
                nc.vector.reciprocal_approx_fast(out=rec_sb[:], in_=den_sb[:])
                bcr_sb = work.tile([HD, 512], F32, tag="bcr")
                nc.gpsimd.partition_broadcast(bcr_sb[:], rec_sb[:])
                nc.vector.tensor_tensor(
                    out=stg[64 * h : 64 * (h + 1), :],
                    in0=ot_ps[0:HD, :],
                    in1=bcr_sb[:],
                    op=mybir.AluOpType.mult,
                )
            nc.sync.dma_start(out=send_d[k].ap(), in_=stg[:])
            nc.gpsimd.collective_compute(
                "AllGather",
                mybir.AluOpType.bypass,
                replica_groups=[list(range(NC_CORES))],
                ins=[send_d[k].ap().opt()],
                outs=[recv_d.ap()[k]],
            )

        # ---- rank-dynamic slice: this core's seq chunk, all 16 heads ----
        psa_cm.__exit__(None, None, None)
        pso = ctx.enter_context(tc.tile_pool(name="pso", bufs=1, space="PSUM"))

        # Wo load: deferred past the preamble so it doesn't compete with xT
        wo_sb = consts.tile([128, NDC, D], BF16, tag="wo")
        for i in range(NDC):
            eng = nc.sync if i % 2 == 0 else nc.scalar
            eng.dma_start(out=wo_sb[:, i, :], in_=wo_e[:, i, :])
        rank = nc.partition_id()
        kidx = nc.s_assert_within(rank >> 1, 0, 3, skip_runtime_assert=True)
        woff = nc.s_assert_within(
            (rank & 1) * SC, 0, SC, skip_runtime_assert=True
        )
        at_sb = work.tile([128, NC_CORES, SC], BF16, tag="at")
        nc.sync.dma_start(
            out=at_sb[:],
            in_=recv_d.ap()[bass.ds(kidx, 1)].rearrange("k j p w -> p (k j) w")[
                :, :, bass.ds(woff, SC)
            ],
        )
        if taps:
            nc.sync.dma_start(out=taps["tap_at"].ap(), in_=at_sb[:])

        # ---- output projection: out[s, :] = attn[s, :] @ Wo ----
        op_ps = pso.tile([128, S], F32, tag="op")  # [128, 2 m x 1024 dout]
        for m in range(SC // 128):
            for j in range(NC_CORES):
                for dn in range(2):
                    nc.tensor.matmul(
                        op_ps[:, 1024 * m + 512 * dn : 1024 * m + 512 * (dn + 1)],
                        lhsT=at_sb[:, j, 128 * m : 128 * (m + 1)],
                        rhs=wo_sb[:, j, 512 * dn : 512 * (dn + 1)],
                        start=(j == 0),
                        stop=(j == NC_CORES - 1),
                    )
        out_sb = work.tile([128, S], F32, tag="osb")
        nc.scalar.copy(out=out_sb[:], in_=op_ps[:])
        nc.sync.dma_start(
            out=out_e.ap().rearrange("(m p) d -> p m d", p=128),
            in_=out_sb[:].rearrange("p (m d) -> p m d", m=SC // 128),
        )


# ---------------- host side ----------------

_CACHE = {}


def _prep_consts():
    # ppm: lhsT of the signed half-swap M (per 64 block: [[0,-I],[I,0]])
    M = np.zeros((128, 128), np.float32)
    for hb in range(2):
        o = 64 * hb
        for j in range(HALF):
            M[o + j, o + HALF + j] = -1.0
            M[o + HALF + j, o + j] = 1.0
    ppm = M.T.astype(np_bf16)
    idm = np.eye(128, dtype=np_bf16)
    # tri[p, j] = 1 if j >= p (valid: sq >= sk within diagonal block)
    tri = (np.arange(128)[None, :] >= np.arange(128)[:, None]).astype(np_bf16)
    return ppm, idm, tri


def kernel(x, rope_cos, rope_sin, Wq, Wk, Wv, Wo):
    if "nc" not in _CACHE:
        _CACHE["nc"] = build_graph()
    nc = _CACHE["nc"]

    x2 = np.asarray(x, np.float32).reshape(S, D)
    xT = np.ascontiguousarray(x2.T).astype(np_bf16)
    cosT = np.asarray(rope_cos, np.float32).T  # [32, S]
    sinT = np.asarray(rope_sin, np.float32).T
    c2 = np.tile(cosT, (4, 1)).astype(np_bf16)  # [128, S]
    s2 = np.tile(sinT, (4, 1)).astype(np_bf16)
    ppm, idm, tri = _prep_consts()

    Wq = np.asarray(Wq, np.float32)
    Wk = np.asarray(Wk, np.float32)
    Wv = np.asarray(Wv, np.float32)
    Wo = np.asarray(Wo, np.float32)

    def chunked(w):  # [1024, X] -> [128, 8, X] (partition-major d-chunks)
        return np.ascontiguousarray(
            w.reshape(NDC, 128, -1).transpose(1, 0, 2)
        ).astype(np_bf16)

    wo_b = chunked(Wo)
    in_maps = []
    for c in range(NC_CORES):
        kv = c // 2
        wq_c = chunked(Wq[:, HPC * HD * c : HPC * HD * (c + 1)])
        wkv_c = chunked(
            np.concatenate(
                [Wk[:, HD * kv : HD * (kv + 1)], Wv[:, HD * kv : HD * (kv + 1)]],
                axis=1,
            )
        )
        in_maps.append(
            {
                "xT": xT,
                "wq": wq_c,
                "wkv": wkv_c,
                "wo": wo_b,
                "c2": c2,
                "s2": s2,
                "ppm": ppm,
                "idm": idm,
                "tri": tri,
            }
        )

    res = run_bass_kernel_spmd(nc, in_maps, core_ids=list(range(NC_CORES)))
    out = np.concatenate([res.results[c]["out"] for c in range(NC_CORES)], axis=0)
    return out.reshape(B, S, D).astype(np.float32)


# revision 54
# speedup vs baseline: 1.0824x; 1.0074x over previous
"""Distributed Trainium2 kernel for GQA attention (nn_Attention_76845554860188).

B=1, S=2048, D=1024, NH=16, NKV=4, HD=64, causal, RoPE, 8 NeuronCores.

Sharding: tensor-parallel over heads. Core c owns q-heads {2c, 2c+1} and their
(shared, GQA) kv-head c//2. Each core projects Q/K/V for all 2048 positions,
runs causal attention for its 2 heads, then an AllToAll redistributes the
per-head outputs into per-sequence shards; each core applies the full output
projection to its 256-row slice. Host concatenates the 8 slices.

On-chip layout is [feature, seq] ("transposed"): scores are computed with keys
on the partition axis, so the softmax denominator comes for free as a ones
column appended to V in the PV matmul. exp() runs on ScalarE with the 1/sqrt(64)
scale folded in; no max-subtraction is needed (logits are O(5) for unit-scale
inputs, far from fp32/bf16 overflow).
"""

import sys

sys.path.insert(0, "/opt/trn_rl_repo")

import numpy as np
import ml_dtypes

import concourse.bass as bass
import concourse.mybir as mybir
import concourse.tile as tile
from concourse import bacc
from concourse.bass_utils import run_bass_kernel_spmd

BF16 = mybir.dt.bfloat16
F32 = mybir.dt.float32

B, S, D = 1, 2048, 1024
NH, NKV, HD = 16, 4, 64
NC_CORES = 8
HPC = NH // NC_CORES  # q heads per core = 2
SC = S // NC_CORES  # seq slice per core = 256
NDC = D // 128  # d chunks = 8
NSB = S // 128  # seq blocks = 16
HALF = HD // 2  # 32

np_bf16 = ml_dtypes.bfloat16


def build_graph(taps=False):
    nc = bacc.Bacc(
        "TRN2", target_bir_lowering=False, debug=False, num_devices=NC_CORES
    )

    # ---- DRAM parameters (per-core shards supplied by host) ----
    xT_e = nc.dram_tensor("xT", [D, S], BF16, kind="ExternalInput")
    # weights arrive pre-rearranged host-side into [partition, d-chunk, cols]
    wq_e = nc.dram_tensor("wq", [128, NDC, HPC * HD], BF16, kind="ExternalInput")
    wkv_e = nc.dram_tensor("wkv", [128, NDC, 2 * HD], BF16, kind="ExternalInput")
    wo_e = nc.dram_tensor("wo", [128, NDC, D], BF16, kind="ExternalInput")
    c2_e = nc.dram_tensor("c2", [128, S], BF16, kind="ExternalInput")
    s2_e = nc.dram_tensor("s2", [128, S], BF16, kind="ExternalInput")
    ppm_e = nc.dram_tensor("ppm", [128, 128], BF16, kind="ExternalInput")
    idm_e = nc.dram_tensor("idm", [128, 128], BF16, kind="ExternalInput")
    tri_e = nc.dram_tensor("tri", [128, 128], BF16, kind="ExternalInput")
    out_e = nc.dram_tensor("out", [SC, D], F32, kind="ExternalOutput")

    # internal DRAM bounce buffers for the per-seq-chunk AllGathers (4 chunks)
    send_d = [nc.dram_tensor(f"ag_send{k}", [128, 512], BF16) for k in range(4)]
    recv_d = nc.dram_tensor(
        "ag_recv", [4, NC_CORES, 128, 512], BF16, addr_space="Shared"
    )
    # tiny warmup collective: absorbs the first-collective setup cost during
    # the preamble so the real AG train isn't delayed by it
    wup_s = nc.dram_tensor("wup_s", [1, 64], BF16)
    wup_r = nc.dram_tensor("wup_r", [NC_CORES, 1, 64], BF16, addr_space="Shared")

    tap_tensors = None
    if taps:
        tap_tensors = {
            "tap_qrot0": nc.dram_tensor("tap_qrot0", [64, S], BF16, kind="ExternalOutput"),
            "tap_qrot1": nc.dram_tensor("tap_qrot1", [64, S], BF16, kind="ExternalOutput"),
            "tap_krot": nc.dram_tensor("tap_krot", [64, S], BF16, kind="ExternalOutput"),
            "tap_vext": nc.dram_tensor("tap_vext", [128, NSB, HD + 1], BF16, kind="ExternalOutput"),
            "tap_pt0": nc.dram_tensor("tap_pt0", [128, S], BF16, kind="ExternalOutput"),
            "tap_ot0": nc.dram_tensor("tap_ot0", [HD + 1, S], F32, kind="ExternalOutput"),
            "tap_stage": nc.dram_tensor("tap_stage", [128, S], BF16, kind="ExternalOutput"),
            "tap_at": nc.dram_tensor("tap_at", [128, NC_CORES, SC], BF16, kind="ExternalOutput"),
        }

    with tile.TileContext(nc) as tc:
        _body(nc, tc, xT_e, wq_e, wkv_e, wo_e, c2_e, s2_e, ppm_e, idm_e, tri_e,
              out_e, send_d, recv_d, wup_s, wup_r, tap_tensors)

    nc.compile()
    return nc


def _body(nc, tc, xT_e, wq_e, wkv_e, wo_e, c2_e, s2_e, ppm_e, idm_e, tri_e,
          out_e, send_d, recv_d, wup_s, wup_r, taps=None):
    from contextlib import ExitStack

    ctx = ExitStack()
    with ctx:
        consts = ctx.enter_context(tc.tile_pool(name="consts", bufs=1))
        work = ctx.enter_context(tc.tile_pool(name="work", bufs=1))
        rope_cm = tc.tile_pool(name="rope", bufs=1)
        rope = rope_cm.__enter__()
        psum_cm = tc.tile_pool(name="psum", bufs=2, space="PSUM")
        psum = psum_cm.__enter__()

        # warmup collective, first in program order
        wup_sb = consts.tile([1, 64], BF16, tag="wup")
        nc.vector.memset(wup_sb[:], 0.0)
        nc.sync.dma_start(out=wup_s.ap(), in_=wup_sb[:])
        nc.gpsimd.collective_compute(
            "AllGather",
            mybir.AluOpType.bypass,
            replica_groups=[list(range(NC_CORES))],
            ins=[wup_s.ap().opt()],
            outs=[wup_r.ap().opt()],
        )

        # ---- load inputs needed by the preamble (Wo is loaded later) ----
        wq_sb = consts.tile([128, NDC, HPC * HD], BF16, tag="wq")
        nc.scalar.dma_start(
            out=wq_sb[:], in_=wq_e.ap()
        )
        wkv_sb = consts.tile([128, NDC, 2 * HD], BF16, tag="wkv")
        nc.scalar.dma_start(
            out=wkv_sb[:], in_=wkv_e.ap()
        )
        xT_sb = consts.tile([128, NDC, S], BF16, tag="xT")
        for i in range(NDC):
            eng = nc.sync if i % 2 == 0 else nc.scalar
            eng.dma_start(out=xT_sb[:, i, :], in_=xT_e[128 * i : 128 * (i + 1), :])
        c2_sb = rope.tile([128, S], BF16, tag="c2")
        nc.sync.dma_start(out=c2_sb[:], in_=c2_e[:, :])
        s2_sb = rope.tile([128, S], BF16, tag="s2")
        nc.scalar.dma_start(out=s2_sb[:], in_=s2_e[:, :])
        ppm_sb = rope.tile([128, 128], BF16, tag="ppm")
        nc.sync.dma_start(out=ppm_sb[:], in_=ppm_e[:, :])
        idm_sb = rope.tile([128, 128], BF16, tag="idm")
        nc.sync.dma_start(out=idm_sb[:], in_=idm_e[:, :])
        tri_sb = consts.tile([128, 128], BF16, tag="tri")
        nc.sync.dma_start(out=tri_sb[:], in_=tri_e[:, :])

        NCH = S // 512  # 512-wide chunks across seq

        # ---- Q/KV projections -> PSUM f32 [128, 2048] ----
        q_ps = psum.tile([128, S], F32, tag="big")
        for i in range(NDC):
            for n in range(NCH):
                nc.tensor.matmul(
                    q_ps[:, 512 * n : 512 * (n + 1)],
                    lhsT=wq_sb[:, i, :],
                    rhs=xT_sb[:, i, 512 * n : 512 * (n + 1)],
                    start=(i == 0),
                    stop=(i == NDC - 1),
                )
        kv_ps = psum.tile([128, S], F32, tag="big")
        for i in range(NDC):
            for n in range(NCH):
                nc.tensor.matmul(
                    kv_ps[:, 512 * n : 512 * (n + 1)],
                    lhsT=wkv_sb[:, i, :],
                    rhs=xT_sb[:, i, 512 * n : 512 * (n + 1)],
                    start=(i == 0),
                    stop=(i == NDC - 1),
                )

        # ---- RoPE on q (both heads at once) ----
        # qrot = (q .* C2) + M @ (q .* S2); lhsT ppm = M.T, idm = I.
        qc_sb = rope.tile([128, S], BF16, tag="qc")
        nc.vector.tensor_tensor(
            out=qc_sb[:], in0=q_ps[:], in1=c2_sb[:], op=mybir.AluOpType.mult
        )
        qs_sb = rope.tile([128, S], BF16, tag="qs")
        nc.vector.tensor_tensor(
            out=qs_sb[:], in0=q_ps[:], in1=s2_sb[:], op=mybir.AluOpType.mult
        )
        qrot_ps = psum.tile([128, S], F32, tag="big")
        for n in range(NCH):
            sl = slice(512 * n, 512 * (n + 1))
            nc.tensor.matmul(
                qrot_ps[:, sl], lhsT=ppm_sb[:], rhs=qs_sb[:, sl], start=True, stop=False
            )
            nc.tensor.matmul(
                qrot_ps[:, sl], lhsT=idm_sb[:], rhs=qc_sb[:, sl], start=False, stop=True
            )
        # per-head tiles (base partition 0) since matmul requires matching bases
        qrot_h = []
        for h in range(HPC):
            qr = work.tile([64, S], BF16, tag=f"qrot{h}")
            nc.scalar.copy(out=qr[:], in_=qrot_ps[64 * h : 64 * (h + 1), :])
            qrot_h.append(qr)
        if taps:
            nc.sync.dma_start(out=taps["tap_qrot0"].ap(), in_=qrot_h[0][:])
            nc.sync.dma_start(out=taps["tap_qrot1"].ap(), in_=qrot_h[1][:])

        # ---- RoPE on k (rows 0:64 of kv), V extraction (rows 64:128) ----
        kc_sb = rope.tile([64, S], BF16, tag="kc")
        nc.vector.tensor_tensor(
            out=kc_sb[:], in0=kv_ps[0:64, :], in1=c2_sb[0:64, :],
            op=mybir.AluOpType.mult,
        )
        ks_sb = rope.tile([64, S], BF16, tag="ks")
        nc.vector.tensor_tensor(
            out=ks_sb[:], in0=kv_ps[0:64, :], in1=s2_sb[0:64, :],
            op=mybir.AluOpType.mult,
        )
        vT_sb = rope.tile([64, S], BF16, tag="vT")
        nc.vector.tensor_copy(out=vT_sb[:], in_=kv_ps[64:128, :])

        krot_ps = psum.tile([128, S], F32, tag="big")
        for n in range(NCH):
            sl = slice(512 * n, 512 * (n + 1))
            nc.tensor.matmul(
                krot_ps[0:64, sl], lhsT=ppm_sb[0:64, 0:64], rhs=ks_sb[:, sl],
                start=True, stop=False,
            )
            nc.tensor.matmul(
                krot_ps[0:64, sl], lhsT=idm_sb[0:64, 0:64], rhs=kc_sb[:, sl],
                start=False, stop=True,
            )
        krot_sb = work.tile([64, S], BF16, tag="krot")
        nc.scalar.copy(out=krot_sb[:], in_=krot_ps[0:64, :])
        if taps:
            nc.sync.dma_start(out=taps["tap_krot"].ap(), in_=krot_sb[:])

        # ---- V transpose: vT [64, S] -> V blocks [128, 64] + ones col ----
        vext_sb = work.tile([128, NSB, HD + 1], BF16, tag="vext")
        nc.vector.memset(vext_sb[:, :, HD : HD + 1], 1.0)
        vt_ps = psum.tile([128, NSB, HD], BF16, tag="big")
        for b in range(NSB):
            nc.tensor.transpose(
                vt_ps[:, b, :], vT_sb[:, 128 * b : 128 * (b + 1)], idm_sb[0:64, 0:64]
            )
        nc.vector.tensor_copy(out=vext_sb[:, :, 0:HD], in_=vt_ps[:])
        if taps:
            nc.sync.dma_start(out=taps["tap_vext"].ap(), in_=vext_sb[:])

        # release RoPE temporaries so their SBUF range can host the PT pool;
        # release the proj-phase PSUM pool so attention can use its own layout
        rope_cm.__exit__(None, None, None)
        psum_cm.__exit__(None, None, None)
        ptp = ctx.enter_context(tc.tile_pool(name="pt", bufs=2, space="SBUF"))
        psa_cm = tc.tile_pool(name="psa", bufs=2, space="PSUM")
        psa = psa_cm.__enter__()

        # ---- attention: sq-chunked; chunk k (both heads) is final after
        # k-block 2k+1 thanks to causality, so its AllGather overlaps the rest
        scale = 1.0 / np.sqrt(HD)
        blk_w = [S - 128 * b for b in range(NSB)]  # ragged valid width per k-block
        blk_off = [0] * NSB
        for b in range(1, NSB):
            blk_off[b] = blk_off[b - 1] + blk_w[b - 1]
        pt_total = blk_off[-1] + blk_w[-1]

        pt_h = [
            ptp.tile([128, pt_total], BF16, tag="ptbuf", name=f"pt_h{h}")
            for h in range(HPC)
        ]

        for b in range(NSB):
            w = blk_w[b]
            kb = krot_sb[:, 128 * b : 128 * (b + 1)]
            for h in range(HPC):
                qh = qrot_h[h][:]
                for n2 in range((w + 1023) // 1024):
                    cw2 = min(1024, w - 1024 * n2)
                    st_ps = psa.tile([128, 1024], F32, tag="st", bufs=3)
                    for n in range((cw2 + 511) // 512):
                        cw = min(512, cw2 - 512 * n)
                        q0 = 128 * b + 1024 * n2 + 512 * n
                        nc.tensor.matmul(
                            st_ps[:, 512 * n : 512 * n + cw],
                            lhsT=kb,
                            rhs=qh[:, q0 : q0 + cw],
                            start=True,
                            stop=True,
                        )
                    nc.scalar.activation(
                        out=pt_h[h][:, blk_off[b] + 1024 * n2 : blk_off[b] + 1024 * n2 + cw2],
                        in_=st_ps[:, 0:cw2],
                        func=mybir.ActivationFunctionType.Exp,
                        scale=scale,
                    )
                # mask the diagonal 128x128 sub-block (sq < sk -> 0)
                nc.vector.tensor_tensor(
                    out=pt_h[h][:, blk_off[b] : blk_off[b] + 128],
                    in0=pt_h[h][:, blk_off[b] : blk_off[b] + 128],
                    in1=tri_sb[:],
                    op=mybir.AluOpType.mult,
                )

            if b % 4 != 3:
                continue
            # chunk k = b//4 (sq 512-wide) is complete: PV + normalize + AllGather
            k = b // 4
            base = 512 * k
            stg = work.tile([128, 512], BF16, tag="stg")
            for h in range(HPC):
                ot_ps = psa.tile([128, 512], F32, tag="ot")
                for b2 in range(b + 1):
                    lo = max(base, 128 * b2)
                    hi = base + 512
                    nc.tensor.matmul(
                        ot_ps[0 : HD + 1, lo - base : hi - base],
                        lhsT=vext_sb[:, b2, :],
                        rhs=pt_h[h][
                            :, blk_off[b2] + lo - 128 * b2 : blk_off[b2] + hi - 128 * b2
                        ],
                        start=(b2 == 0),
                        stop=(b2 == b),
                    )
                den_sb = work.tile([1, 512], F32, tag="den")
                nc.vector.tensor_copy(out=den_sb[:], in_=ot_ps[HD : HD + 1, :])
                rec_sb = work.tile([1, 512], F32, tag="rec")
                nc.vector.reciprocal_approx_fast(out=rec_sb[:], in_=den_sb[:])
                bcr_sb = work.tile([HD, 512], F32, tag="bcr")
                nc.gpsimd.partition_broadcast(bcr_sb[:], rec_sb[:])
                nc.vector.tensor_tensor(
                    out=stg[64 * h : 64 * (h + 1), :],
                    in0=ot_ps[0:HD, :],
                    in1=bcr_sb[:],
                    op=mybir.AluOpType.mult,
                )
            nc.sync.dma_start(out=send_d[k].ap(), in_=stg[:])
            nc.gpsimd.collective_compute(
                "AllGather",
                mybir.AluOpType.bypass,
                replica_groups=[list(range(NC_CORES))],
                ins=[send_d[k].ap().opt()],
                outs=[recv_d.ap()[k]],
            )

        # ---- rank-dynamic slice: this core's seq chunk, all 16 heads ----
        psa_cm.__exit__(None, None, None)
        pso = ctx.enter_context(tc.tile_pool(name="pso", bufs=1, space="PSUM"))

        # Wo load: deferred past the preamble so it doesn't compete with xT
        wo_sb = consts.tile([128, NDC, D], BF16, tag="wo")
        for i in range(NDC):
            eng = nc.sync if i % 2 == 0 else nc.scalar
            eng.dma_start(out=wo_sb[:, i, :], in_=wo_e[:, i, :])
        rank = nc.partition_id()
        kidx = nc.s_assert_within(rank >> 1, 0, 3, skip_runtime_assert=True)
        woff = nc.s_assert_within(
            (rank & 1) * SC, 0, SC, skip_runtime_assert=True
        )
        at_sb = work.tile([128, NC_CORES, SC], BF16, tag="at")
        nc.sync.dma_start(
            out=at_sb[:],
            in_=recv_d.ap()[bass.ds(kidx, 1)].rearrange("k j p w -> p (k j) w")[
                :, :, bass.ds(woff, SC)
            ],
        )
        if taps:
            nc.sync.dma_start(out=taps["tap_at"].ap(), in_=at_sb[:])

        # ---- output projection: out[s, :] = attn[s, :] @ Wo ----
        op_ps = pso.tile([128, S], F32, tag="op")  # [128, 2 m x 1024 dout]
        for m in range(SC // 128):
            for j in range(NC_CORES):
                for dn in range(2):
                    nc.tensor.matmul(
                        op_ps[:, 1024 * m + 512 * dn : 1024 * m + 512 * (dn + 1)],
                        lhsT=at_sb[:, j, 128 * m : 128 * (m + 1)],
                        rhs=wo_sb[:, j, 512 * dn : 512 * (dn + 1)],
                        start=(j == 0),
                        stop=(j == NC_CORES - 1),
                    )
        out_sb = work.tile([128, S], F32, tag="osb")
        nc.scalar.copy(out=out_sb[:], in_=op_ps[:])
        nc.sync.dma_start(
            out=out_e.ap().rearrange("(m p) d -> p m d", p=128),
            in_=out_sb[:].rearrange("p (m d) -> p m d", m=SC // 128),
        )


# ---------------- host side ----------------

_CACHE = {}


def _prep_consts():
    # ppm: lhsT of the signed half-swap M (per 64 block: [[0,-I],[I,0]])
    M = np.zeros((128, 128), np.float32)
    for hb in range(2):
        o = 64 * hb
        for j in range(HALF):
            M[o + j, o + HALF + j] = -1.0
            M[o + HALF + j, o + j] = 1.0
    ppm = M.T.astype(np_bf16)
    idm = np.eye(128, dtype=np_bf16)
    # tri[p, j] = 1 if j >= p (valid: sq >= sk within diagonal block)
    tri = (np.arange(128)[None, :] >= np.arange(128)[:, None]).astype(np_bf16)
    return ppm, idm, tri


def kernel(x, rope_cos, rope_sin, Wq, Wk, Wv, Wo):
    if "nc" not in _CACHE:
        _CACHE["nc"] = build_graph()
    nc = _CACHE["nc"]

    x2 = np.asarray(x, np.float32).reshape(S, D)
    xT = np.ascontiguousarray(x2.T).astype(np_bf16)
    cosT = np.asarray(rope_cos, np.float32).T  # [32, S]
    sinT = np.asarray(rope_sin, np.float32).T
    c2 = np.tile(cosT, (4, 1)).astype(np_bf16)  # [128, S]
    s2 = np.tile(sinT, (4, 1)).astype(np_bf16)
    ppm, idm, tri = _prep_consts()

    Wq = np.asarray(Wq, np.float32)
    Wk = np.asarray(Wk, np.float32)
    Wv = np.asarray(Wv, np.float32)
    Wo = np.asarray(Wo, np.float32)

    def chunked(w):  # [1024, X] -> [128, 8, X] (partition-major d-chunks)
        return np.ascontiguousarray(
            w.reshape(NDC, 128, -1).transpose(1, 0, 2)
        ).astype(np_bf16)

    wo_b = chunked(Wo)
    in_maps = []
    for c in range(NC_CORES):
        kv = c // 2
        wq_c = chunked(Wq[:, HPC * HD * c : HPC * HD * (c + 1)])
        wkv_c = chunked(
            np.concatenate(
                [Wk[:, HD * kv : HD * (kv + 1)], Wv[:, HD * kv : HD * (kv + 1)]],
                axis=1,
            )
        )
        in_maps.append(
            {
                "xT": xT,
                "wq": wq_c,
                "wkv": wkv_c,
                "wo": wo_b,
                "c2": c2,
                "s2": s2,
                "ppm": ppm,
                "idm": idm,
                "tri": tri,
            }
        )

    res = run_bass_kernel_spmd(nc, in_maps, core_ids=list(range(NC_CORES)))
    out = np.concatenate([res.results[c]["out"] for c in range(NC_CORES)], axis=0)
    return out.reshape(B, S, D).astype(np.float32)


# revision 62
# speedup vs baseline: 1.1963x; 1.1052x over previous
"""Distributed Trainium2 kernel for GQA attention (nn_Attention_76845554860188).

B=1, S=2048, D=1024, NH=16, NKV=4, HD=64, causal, RoPE, 8 NeuronCores.

Sharding: tensor-parallel over heads. Core c owns q-heads {2c, 2c+1} and their
(shared, GQA) kv-head c//2. Each core projects Q/K/V for all 2048 positions,
runs causal attention for its 2 heads, then an AllToAll redistributes the
per-head outputs into per-sequence shards; each core applies the full output
projection to its 256-row slice. Host concatenates the 8 slices.

On-chip layout is [feature, seq] ("transposed"): scores are computed with keys
on the partition axis, so the softmax denominator comes for free as a ones
column appended to V in the PV matmul. exp() runs on ScalarE with the 1/sqrt(64)
scale folded in; no max-subtraction is needed (logits are O(5) for unit-scale
inputs, far from fp32/bf16 overflow).
"""

import sys

sys.path.insert(0, "/opt/trn_rl_repo")

import numpy as np
import ml_dtypes

import concourse.bass as bass
import concourse.mybir as mybir
import concourse.tile as tile
from concourse import bacc
from concourse.bass_utils import run_bass_kernel_spmd

BF16 = mybir.dt.bfloat16
F32 = mybir.dt.float32

B, S, D = 1, 2048, 1024
NH, NKV, HD = 16, 4, 64
NC_CORES = 8
HPC = NH // NC_CORES  # q heads per core = 2
SC = S // NC_CORES  # seq slice per core = 256
NDC = D // 128  # d chunks = 8
NSB = S // 128  # seq blocks = 16
HALF = HD // 2  # 32

np_bf16 = ml_dtypes.bfloat16


def build_graph(taps=False):
    nc = bacc.Bacc(
        "TRN2", target_bir_lowering=False, debug=False, num_devices=NC_CORES
    )

    # ---- DRAM parameters (per-core shards supplied by host) ----
    xT_e = nc.dram_tensor("xT", [D, S], BF16, kind="ExternalInput")
    # weights arrive pre-rearranged host-side into [partition, d-chunk, cols]
    wq_e = nc.dram_tensor("wq", [128, NDC, HPC * HD], BF16, kind="ExternalInput")
    wkv_e = nc.dram_tensor("wkv", [128, NDC, 2 * HD], BF16, kind="ExternalInput")
    wo_e = nc.dram_tensor("wo", [128, NDC, D], BF16, kind="ExternalInput")
    c2_e = nc.dram_tensor("c2", [128, S], BF16, kind="ExternalInput")
    s2_e = nc.dram_tensor("s2", [128, S], BF16, kind="ExternalInput")
    ppm_e = nc.dram_tensor("ppm", [128, 128], BF16, kind="ExternalInput")
    idm_e = nc.dram_tensor("idm", [128, 128], BF16, kind="ExternalInput")
    tri_e = nc.dram_tensor("tri", [128, 128], BF16, kind="ExternalInput")
    out_e = nc.dram_tensor("out", [SC, D], F32, kind="ExternalOutput")

    # internal DRAM bounce buffers for the per-seq-chunk AllGathers (4 chunks)
    send_d = [nc.dram_tensor(f"ag_send{k}", [128, 512], BF16) for k in range(4)]
    recv_d = nc.dram_tensor(
        "ag_recv", [4, NC_CORES, 128, 512], BF16, addr_space="Shared"
    )
    # tiny warmup collective: absorbs the first-collective setup cost during
    # the preamble so the real AG train isn't delayed by it
    wup_s = nc.dram_tensor("wup_s", [1, 64], BF16)
    wup_r = nc.dram_tensor("wup_r", [NC_CORES, 1, 64], BF16, addr_space="Shared")

    tap_tensors = None
    if taps:
        tap_tensors = {
            "tap_qrot0": nc.dram_tensor("tap_qrot0", [64, S], BF16, kind="ExternalOutput"),
            "tap_qrot1": nc.dram_tensor("tap_qrot1", [64, S], BF16, kind="ExternalOutput"),
            "tap_krot": nc.dram_tensor("tap_krot", [64, S], BF16, kind="ExternalOutput"),
            "tap_vext": nc.dram_tensor("tap_vext", [128, NSB, HD + 1], BF16, kind="ExternalOutput"),
            "tap_pt0": nc.dram_tensor("tap_pt0", [128, S], BF16, kind="ExternalOutput"),
            "tap_ot0": nc.dram_tensor("tap_ot0", [HD + 1, S], F32, kind="ExternalOutput"),
            "tap_stage": nc.dram_tensor("tap_stage", [128, S], BF16, kind="ExternalOutput"),
            "tap_at": nc.dram_tensor("tap_at", [128, NC_CORES, SC], BF16, kind="ExternalOutput"),
        }

    with tile.TileContext(nc) as tc:
        _body(nc, tc, xT_e, wq_e, wkv_e, wo_e, c2_e, s2_e, ppm_e, idm_e, tri_e,
              out_e, send_d, recv_d, wup_s, wup_r, tap_tensors)

    nc.compile()
    return nc


def _body(nc, tc, xT_e, wq_e, wkv_e, wo_e, c2_e, s2_e, ppm_e, idm_e, tri_e,
          out_e, send_d, recv_d, wup_s, wup_r, taps=None):
    from contextlib import ExitStack

    ctx = ExitStack()
    with ctx:
        consts = ctx.enter_context(tc.tile_pool(name="consts", bufs=1))
        work = ctx.enter_context(tc.tile_pool(name="work", bufs=1))
        rope_cm = tc.tile_pool(name="rope", bufs=1)
        rope = rope_cm.__enter__()
        psum_cm = tc.tile_pool(name="psum", bufs=2, space="PSUM")
        psum = psum_cm.__enter__()

        # warmup collective, first in program order
        wup_sb = consts.tile([1, 64], BF16, tag="wup")
        nc.vector.memset(wup_sb[:], 0.0)
        nc.sync.dma_start(out=wup_s.ap(), in_=wup_sb[:])
        nc.gpsimd.collective_compute(
            "AllGather",
            mybir.AluOpType.bypass,
            replica_groups=[list(range(NC_CORES))],
            ins=[wup_s.ap().opt()],
            outs=[wup_r.ap().opt()],
        )

        # ---- load inputs needed by the preamble (Wo is loaded later) ----
        wq_sb = consts.tile([128, NDC, HPC * HD], BF16, tag="wq")
        nc.scalar.dma_start(
            out=wq_sb[:], in_=wq_e.ap()
        )
        wkv_sb = consts.tile([128, NDC, 2 * HD], BF16, tag="wkv")
        nc.scalar.dma_start(
            out=wkv_sb[:], in_=wkv_e.ap()
        )
        xT_sb = consts.tile([128, NDC, S], BF16, tag="xT")
        for i in range(NDC):
            eng = nc.sync if i % 2 == 0 else nc.scalar
            eng.dma_start(out=xT_sb[:, i, :], in_=xT_e[128 * i : 128 * (i + 1), :])
        c2_sb = rope.tile([128, S], BF16, tag="c2")
        nc.sync.dma_start(out=c2_sb[:], in_=c2_e[:, :])
        s2_sb = rope.tile([128, S], BF16, tag="s2")
        nc.scalar.dma_start(out=s2_sb[:], in_=s2_e[:, :])
        ppm_sb = rope.tile([128, 128], BF16, tag="ppm")
        nc.sync.dma_start(out=ppm_sb[:], in_=ppm_e[:, :])
        idm_sb = rope.tile([128, 128], BF16, tag="idm")
        nc.sync.dma_start(out=idm_sb[:], in_=idm_e[:, :])
        tri_sb = consts.tile([128, 128], BF16, tag="tri")
        nc.sync.dma_start(out=tri_sb[:], in_=tri_e[:, :])

        NCH = S // 512  # 512-wide chunks across seq

        # ---- Q/KV projections -> PSUM f32 [128, 2048] ----
        q_ps = psum.tile([128, S], F32, tag="big")
        for i in range(NDC):
            for n in range(NCH):
                nc.tensor.matmul(
                    q_ps[:, 512 * n : 512 * (n + 1)],
                    lhsT=wq_sb[:, i, :],
                    rhs=xT_sb[:, i, 512 * n : 512 * (n + 1)],
                    start=(i == 0),
                    stop=(i == NDC - 1),
                )
        kv_ps = psum.tile([128, S], F32, tag="big")
        for i in range(NDC):
            for n in range(NCH):
                nc.tensor.matmul(
                    kv_ps[:, 512 * n : 512 * (n + 1)],
                    lhsT=wkv_sb[:, i, :],
                    rhs=xT_sb[:, i, 512 * n : 512 * (n + 1)],
                    start=(i == 0),
                    stop=(i == NDC - 1),
                )

        # ---- RoPE on q (both heads at once) ----
        # qrot = (q .* C2) + M @ (q .* S2); lhsT ppm = M.T, idm = I.
        qc_sb = rope.tile([128, S], BF16, tag="qc")
        nc.vector.tensor_tensor(
            out=qc_sb[:], in0=q_ps[:], in1=c2_sb[:], op=mybir.AluOpType.mult
        )
        qs_sb = rope.tile([128, S], BF16, tag="qs")
        nc.vector.tensor_tensor(
            out=qs_sb[:], in0=q_ps[:], in1=s2_sb[:], op=mybir.AluOpType.mult
        )
        qrot_ps = psum.tile([128, S], F32, tag="big")
        for n in range(NCH):
            sl = slice(512 * n, 512 * (n + 1))
            nc.tensor.matmul(
                qrot_ps[:, sl], lhsT=ppm_sb[:], rhs=qs_sb[:, sl], start=True, stop=False
            )
            nc.tensor.matmul(
                qrot_ps[:, sl], lhsT=idm_sb[:], rhs=qc_sb[:, sl], start=False, stop=True
            )
        # per-head tiles (base partition 0) since matmul requires matching bases
        qrot_h = []
        for h in range(HPC):
            qr = work.tile([64, S], BF16, tag=f"qrot{h}")
            nc.scalar.copy(out=qr[:], in_=qrot_ps[64 * h : 64 * (h + 1), :])
            qrot_h.append(qr)
        if taps:
            nc.sync.dma_start(out=taps["tap_qrot0"].ap(), in_=qrot_h[0][:])
            nc.sync.dma_start(out=taps["tap_qrot1"].ap(), in_=qrot_h[1][:])

        # ---- RoPE on k (rows 0:64 of kv), V extraction (rows 64:128) ----
        kc_sb = rope.tile([64, S], BF16, tag="kc")
        nc.vector.tensor_tensor(
            out=kc_sb[:], in0=kv_ps[0:64, :], in1=c2_sb[0:64, :],
            op=mybir.AluOpType.mult,
        )
        ks_sb = rope.tile([64, S], BF16, tag="ks")
        nc.vector.tensor_tensor(
            out=ks_sb[:], in0=kv_ps[0:64, :], in1=s2_sb[0:64, :],
            op=mybir.AluOpType.mult,
        )
        vT_sb = rope.tile([64, S], BF16, tag="vT")
        nc.vector.tensor_copy(out=vT_sb[:], in_=kv_ps[64:128, :])

        krot_ps = psum.tile([128, S], F32, tag="big")
        for n in range(NCH):
            sl = slice(512 * n, 512 * (n + 1))
            nc.tensor.matmul(
                krot_ps[0:64, sl], lhsT=ppm_sb[0:64, 0:64], rhs=ks_sb[:, sl],
                start=True, stop=False,
            )
            nc.tensor.matmul(
                krot_ps[0:64, sl], lhsT=idm_sb[0:64, 0:64], rhs=kc_sb[:, sl],
                start=False, stop=True,
            )
        krot_sb = work.tile([64, S], BF16, tag="krot")
        nc.scalar.copy(out=krot_sb[:], in_=krot_ps[0:64, :])
        if taps:
            nc.sync.dma_start(out=taps["tap_krot"].ap(), in_=krot_sb[:])

        # ---- V transpose: vT [64, S] -> V blocks [128, 64] + ones col ----
        vext_sb = work.tile([128, NSB, HD + 1], BF16, tag="vext")
        nc.vector.memset(vext_sb[:, :, HD : HD + 1], 1.0)
        vt_ps = psum.tile([128, NSB, HD], BF16, tag="big")
        for b in range(NSB):
            nc.tensor.transpose(
                vt_ps[:, b, :], vT_sb[:, 128 * b : 128 * (b + 1)], idm_sb[0:64, 0:64]
            )
        nc.vector.tensor_copy(out=vext_sb[:, :, 0:HD], in_=vt_ps[:])
        if taps:
            nc.sync.dma_start(out=taps["tap_vext"].ap(), in_=vext_sb[:])

        # release RoPE temporaries so their SBUF range can host the PT pool;
        # release the proj-phase PSUM pool so attention can use its own layout
        rope_cm.__exit__(None, None, None)
        psum_cm.__exit__(None, None, None)
        ptp = ctx.enter_context(tc.tile_pool(name="pt", bufs=2, space="SBUF"))
        psa_cm = tc.tile_pool(name="psa", bufs=2, space="PSUM")
        psa = psa_cm.__enter__()

        # ---- attention: sq-chunked; chunk k (both heads) is final after
        # k-block 2k+1 thanks to causality, so its AllGather overlaps the rest
        scale = 1.0 / np.sqrt(HD)
        blk_w = [S - 128 * b for b in range(NSB)]  # ragged valid width per k-block
        blk_off = [0] * NSB
        for b in range(1, NSB):
            blk_off[b] = blk_off[b - 1] + blk_w[b - 1]
        pt_total = blk_off[-1] + blk_w[-1]

        pt_h = [
            ptp.tile([128, pt_total], BF16, tag="ptbuf", name=f"pt_h{h}")
            for h in range(HPC)
        ]

        for b in range(NSB):
            w = blk_w[b]
            kb = krot_sb[:, 128 * b : 128 * (b + 1)]
            for h in range(HPC):
                qh = qrot_h[h][:]
                for n2 in range((w + 1023) // 1024):
                    cw2 = min(1024, w - 1024 * n2)
                    st_ps = psa.tile([128, 1024], F32, tag="st", bufs=3)
                    for n in range((cw2 + 511) // 512):
                        cw = min(512, cw2 - 512 * n)
                        q0 = 128 * b + 1024 * n2 + 512 * n
                        nc.tensor.matmul(
                            st_ps[:, 512 * n : 512 * n + cw],
                            lhsT=kb,
                            rhs=qh[:, q0 : q0 + cw],
                            start=True,
                            stop=True,
                        )
                    nc.scalar.activation(
                        out=pt_h[h][
                            :, blk_off[b] + 1024 * n2 : blk_off[b] + 1024 * n2 + cw2
                        ],
                        in_=st_ps[:, 0:cw2],
                        func=mybir.ActivationFunctionType.Exp,
                        scale=scale,
                    )
                # mask the diagonal 128x128 sub-block (sq < sk -> 0)
                nc.vector.tensor_tensor(
                    out=pt_h[h][:, blk_off[b] : blk_off[b] + 128],
                    in0=pt_h[h][:, blk_off[b] : blk_off[b] + 128],
                    in1=tri_sb[:],
                    op=mybir.AluOpType.mult,
                )

            if b % 4 != 3:
                continue
            # chunk k = b//4 (sq 512-wide) is complete: PV + normalize + AllGather
            k = b // 4
            base = 512 * k
            stg = work.tile([128, 512], BF16, tag="stg")
            for h in range(HPC):
                ot_ps = psa.tile([128, 512], F32, tag="ot")
                for b2 in range(b + 1):
                    lo = max(base, 128 * b2)
                    hi = base + 512
                    nc.tensor.matmul(
                        ot_ps[0 : HD + 1, lo - base : hi - base],
                        lhsT=vext_sb[:, b2, :],
                        rhs=pt_h[h][
                            :, blk_off[b2] + lo - 128 * b2 : blk_off[b2] + hi - 128 * b2
                        ],
                        start=(b2 == 0),
                        stop=(b2 == b),
                    )
                den_sb = work.tile([1, 512], F32, tag="den")
                nc.vector.tensor_copy(out=den_sb[:], in_=ot_ps[HD : HD + 1, :])
                rec_sb = work.tile([1, 512], F32, tag="rec")
                nc.vector.reciprocal_approx_fast(out=rec_sb[:], in_=den_sb[:])
                bcr_sb = work.tile([HD, 512], F32, tag="bcr")
                nc.gpsimd.partition_broadcast(bcr_sb[:], rec_sb[:])
                nc.vector.tensor_tensor(
                    out=stg[64 * h : 64 * (h + 1), :],
                    in0=ot_ps[0:HD, :],
                    in1=bcr_sb[:],
                    op=mybir.AluOpType.mult,
                )
            nc.sync.dma_start(out=send_d[k].ap(), in_=stg[:])
            nc.gpsimd.collective_compute(
                "AllGather",
                mybir.AluOpType.bypass,
                replica_groups=[list(range(NC_CORES))],
                ins=[send_d[k].ap().opt()],
                outs=[recv_d.ap()[k]],
            )

        # ---- rank-dynamic slice: this core's seq chunk, all 16 heads ----
        psa_cm.__exit__(None, None, None)
        pso = ctx.enter_context(tc.tile_pool(name="pso", bufs=1, space="PSUM"))

        # Wo load: deferred past the preamble so it doesn't compete with xT
        wo_sb = consts.tile([128, NDC, D], BF16, tag="wo")
        for i in range(NDC):
            eng = nc.sync if i % 2 == 0 else nc.scalar
            eng.dma_start(out=wo_sb[:, i, :], in_=wo_e[:, i, :])
        rank = nc.partition_id()
        kidx = nc.s_assert_within(rank >> 1, 0, 3, skip_runtime_assert=True)
        woff = nc.s_assert_within(
            (rank & 1) * SC, 0, SC, skip_runtime_assert=True
        )
        at_sb = work.tile([128, NC_CORES, SC], BF16, tag="at")
        nc.sync.dma_start(
            out=at_sb[:],
            in_=recv_d.ap()[bass.ds(kidx, 1)].rearrange("k j p w -> p (k j) w")[
                :, :, bass.ds(woff, SC)
            ],
        )
        if taps:
            nc.sync.dma_start(out=taps["tap_at"].ap(), in_=at_sb[:])

        # ---- output projection: out[s, :] = attn[s, :] @ Wo ----
        op_ps = pso.tile([128, S], F32, tag="op")  # [128, 2 m x 1024 dout]
        for m in range(SC // 128):
            for j in range(NC_CORES):
                for dn in range(2):
                    nc.tensor.matmul(
                        op_ps[:, 1024 * m + 512 * dn : 1024 * m + 512 * (dn + 1)],
                        lhsT=at_sb[:, j, 128 * m : 128 * (m + 1)],
                        rhs=wo_sb[:, j, 512 * dn : 512 * (dn + 1)],
                        start=(j == 0),
                        stop=(j == NC_CORES - 1),
                    )
        out_sb = work.tile([128, S], F32, tag="osb")
        nc.scalar.copy(out=out_sb[:], in_=op_ps[:])
        nc.sync.dma_start(
            out=out_e.ap().rearrange("(m p) d -> p m d", p=128),
            in_=out_sb[:].rearrange("p (m d) -> p m d", m=SC // 128),
        )


# ---------------- host side ----------------

_CACHE = {}


def _prep_consts():
    # ppm: lhsT of the signed half-swap M (per 64 block: [[0,-I],[I,0]])
    M = np.zeros((128, 128), np.float32)
    for hb in range(2):
        o = 64 * hb
        for j in range(HALF):
            M[o + j, o + HALF + j] = -1.0
            M[o + HALF + j, o + j] = 1.0
    ppm = M.T.astype(np_bf16)
    idm = np.eye(128, dtype=np_bf16)
    # tri[p, j] = 1 if j >= p (valid: sq >= sk within diagonal block)
    tri = (np.arange(128)[None, :] >= np.arange(128)[:, None]).astype(np_bf16)
    return ppm, idm, tri


def kernel(x, rope_cos, rope_sin, Wq, Wk, Wv, Wo):
    if "nc" not in _CACHE:
        _CACHE["nc"] = build_graph()
    nc = _CACHE["nc"]

    x2 = np.asarray(x, np.float32).reshape(S, D)
    xT = np.ascontiguousarray(x2.T).astype(np_bf16)
    cosT = np.asarray(rope_cos, np.float32).T  # [32, S]
    sinT = np.asarray(rope_sin, np.float32).T
    c2 = np.tile(cosT, (4, 1)).astype(np_bf16)  # [128, S]
    s2 = np.tile(sinT, (4, 1)).astype(np_bf16)
    ppm, idm, tri = _prep_consts()

    Wq = np.asarray(Wq, np.float32)
    Wk = np.asarray(Wk, np.float32)
    Wv = np.asarray(Wv, np.float32)
    Wo = np.asarray(Wo, np.float32)

    def chunked(w):  # [1024, X] -> [128, 8, X] (partition-major d-chunks)
        return np.ascontiguousarray(
            w.reshape(NDC, 128, -1).transpose(1, 0, 2)
        ).astype(np_bf16)

    wo_b = chunked(Wo)
    in_maps = []
    for c in range(NC_CORES):
        kv = c // 2
        wq_c = chunked(Wq[:, HPC * HD * c : HPC * HD * (c + 1)])
        wkv_c = chunked(
            np.concatenate(
                [Wk[:, HD * kv : HD * (kv + 1)], Wv[:, HD * kv : HD * (kv + 1)]],
                axis=1,
            )
        )
        in_maps.append(
            {
                "xT": xT,
                "wq": wq_c,
                "wkv": wkv_c,
                "wo": wo_b,
                "c2": c2,
                "s2": s2,
                "ppm": ppm,
                "idm": idm,
                "tri": tri,
            }
        )

    res = run_bass_kernel_spmd(nc, in_maps, core_ids=list(range(NC_CORES)))
    out = np.concatenate([res.results[c]["out"] for c in range(NC_CORES)], axis=0)
    return out.reshape(B, S, D).astype(np.float32)
